# revision 10
# baseline (speedup 1.0000x reference)
"""Grouped single-step GRU (16 independent GRU cells), Trainium2 Bass kernel.

Problem shapes (hardcoded): B=8192, U=16, I=H=128, fp32.
  gx = einsum("bui,ugi->bug", x, w_ih) + b_ih
  gh = einsum("buh,ugh->bug", h, w_hh) + b_hh
  r = sig(gx_r + gh_r); z = sig(gx_z + gh_z); n = tanh(gx_n + r * gh_n)
  out = (1-z)*n + z*h

Sharding: expert/unit-parallel — each of the 8 cores owns 2 units and the
full batch. This avoids replicating weights (only 0.75 MB of weights per
core) so per-core HBM traffic is ~24.75 MB, the problem's memory floor.

On-chip layout: gate dim (128) on partitions, batch on the free dim.
Host pre-transposes x/h to [u, feat, batch] so the contraction dim (feat)
lands on partitions for the TensorE moving operand; weights are
pre-transposed to [u, feat, 3H] to serve as stationary operands.
r/z gates accumulate their x- and h- matmuls in PSUM (2 matmuls, one bank);
the n gate keeps xn/hn separate (r gates hn before the sum).
"""

import os
import sys

import numpy as np

B, U, I, H = 8192, 16, 128, 128
N_CORES = 8
U_LOC = U // N_CORES  # units per core
BT = 512              # batch tile (free dim; one PSUM bank in fp32)
NT = B // BT

_CACHE = {}


def _import_concourse():
    try:
        import concourse.bass  # noqa: F401
    except ImportError:
        for p in ("/opt/trn_rl_repo", "/root/.axon_site/_ro/trn_rl_repo"):
            if os.path.isdir(p) and p not in sys.path:
                sys.path.insert(0, p)
        import concourse.bass  # noqa: F401


def _build():
    if "nc" in _CACHE:
        return _CACHE["nc"]
    _import_concourse()
    from contextlib import ExitStack

    import concourse.bacc as bacc
    import concourse.bass as bass
    import concourse.tile as tile
    from concourse import mybir

    f32 = mybir.dt.float32
    AFT = mybir.ActivationFunctionType
    ALU = mybir.AluOpType

    nc = bacc.Bacc(None, target_bir_lowering=False)
    x_t = nc.declare_dram_parameter("x_t", [U_LOC, I, B], f32, isOutput=False)
    h_t = nc.declare_dram_parameter("h_t", [U_LOC, H, B], f32, isOutput=False)
    wih = nc.declare_dram_parameter("wih", [U_LOC, I, 3 * H], f32, isOutput=False)
    whh = nc.declare_dram_parameter("whh", [U_LOC, H, 3 * H], f32, isOutput=False)
    bia = nc.declare_dram_parameter("bia", [H, U_LOC, 4], f32, isOutput=False)
    out_t = nc.declare_dram_parameter("out_t", [U_LOC, H, B], f32, isOutput=True)

    with ExitStack() as ctx:
        tc = ctx.enter_context(tile.TileContext(nc))
        wpool = ctx.enter_context(tc.tile_pool(name="w", bufs=1))
        xpool = ctx.enter_context(tc.tile_pool(name="x", bufs=4))
        hpool = ctx.enter_context(tc.tile_pool(name="h", bufs=4))
        tmp = ctx.enter_context(tc.tile_pool(name="tmp", bufs=3))
        opool = ctx.enter_context(tc.tile_pool(name="o", bufs=4))
        psum = ctx.enter_context(tc.tile_pool(name="psum", bufs=2, space="PSUM"))

        w_ih_sb = wpool.tile([I, U_LOC, 3 * H], f32)
        w_hh_sb = wpool.tile([H, U_LOC, 3 * H], f32)
        bias_sb = wpool.tile([H, U_LOC, 4], f32)
        nc.sync.dma_start(out=w_ih_sb, in_=wih[:].rearrange("u i g -> i u g"))
        nc.sync.dma_start(out=w_hh_sb, in_=whh[:].rearrange("u i g -> i u g"))
        nc.sync.dma_start(out=bias_sb, in_=bia[:])

        for u in range(U_LOC):
            for t in range(NT):
                bs = slice(t * BT, (t + 1) * BT)
                x_sb = xpool.tile([I, BT], f32, tag="x")
                h_sb = hpool.tile([H, BT], f32, tag="h")
                nc.sync.dma_start(out=x_sb, in_=x_t[u, :, bs])
                nc.sync.dma_start(out=h_sb, in_=h_t[u, :, bs])

                p_r = psum.tile([H, BT], f32, tag="pr")
                p_z = psum.tile([H, BT], f32, tag="pz")
                p_xn = psum.tile([H, BT], f32, tag="pxn")
                p_hn = psum.tile([H, BT], f32, tag="phn")

                # r/z: accumulate x- and h-contributions in PSUM.
                nc.tensor.matmul(p_r, w_ih_sb[:, u, 0:H], x_sb, start=True, stop=False)
                nc.tensor.matmul(p_r, w_hh_sb[:, u, 0:H], h_sb, start=False, stop=True)
                nc.tensor.matmul(p_z, w_ih_sb[:, u, H:2 * H], x_sb, start=True, stop=False)
                nc.tensor.matmul(p_z, w_hh_sb[:, u, H:2 * H], h_sb, start=False, stop=True)
                nc.tensor.matmul(p_xn, w_ih_sb[:, u, 2 * H:], x_sb, start=True, stop=True)
                nc.tensor.matmul(p_hn, w_hh_sb[:, u, 2 * H:], h_sb, start=True, stop=True)

                r_sb = tmp.tile([H, BT], f32, tag="r")
                z_sb = tmp.tile([H, BT], f32, tag="z")
                nc.scalar.activation(out=r_sb, in_=p_r, func=AFT.Sigmoid,
                                     bias=bias_sb[:, u, 0:1])
                nc.scalar.activation(out=z_sb, in_=p_z, func=AFT.Sigmoid,
                                     bias=bias_sb[:, u, 1:2])
                # m = (hn + b_hhn) * r
                m_sb = tmp.tile([H, BT], f32, tag="m")
                nc.vector.scalar_tensor_tensor(
                    out=m_sb, in0=p_hn, scalar=bias_sb[:, u, 3:4], in1=r_sb,
                    op0=ALU.add, op1=ALU.mult)
                s_sb = tmp.tile([H, BT], f32, tag="s")
                nc.vector.tensor_add(s_sb, m_sb, p_xn)
                n_sb = tmp.tile([H, BT], f32, tag="n")
                nc.scalar.activation(out=n_sb, in_=s_sb, func=AFT.Tanh,
                                     bias=bias_sb[:, u, 2:3])
                # out = n + z*(h - n)
                d_sb = tmp.tile([H, BT], f32, tag="d")
                nc.vector.tensor_sub(d_sb, h_sb, n_sb)
                zd_sb = tmp.tile([H, BT], f32, tag="zd")
                nc.vector.tensor_mul(zd_sb, z_sb, d_sb)
                o_sb = opool.tile([H, BT], f32, tag="o")
                nc.vector.tensor_add(o_sb, n_sb, zd_sb)
                nc.sync.dma_start(out=out_t[u, :, bs], in_=o_sb)

    nc.compile()
    _CACHE["nc"] = nc
    return nc


def _make_in_maps(inputs, hidden, w_ih, w_hh, b_ih, b_hh):
    x_all = np.ascontiguousarray(inputs.transpose(1, 2, 0), dtype=np.float32)
    h_all = np.ascontiguousarray(hidden.transpose(1, 2, 0), dtype=np.float32)
    wihT = np.ascontiguousarray(w_ih.transpose(0, 2, 1), dtype=np.float32)
    whhT = np.ascontiguousarray(w_hh.transpose(0, 2, 1), dtype=np.float32)
    bias_r = (b_ih[:, :H] + b_hh[:, :H]).astype(np.float32)
    bias_z = (b_ih[:, H:2 * H] + b_hh[:, H:2 * H]).astype(np.float32)
    b_ihn = b_ih[:, 2 * H:].astype(np.float32)
    b_hhn = b_hh[:, 2 * H:].astype(np.float32)
    in_maps = []
    for c in range(N_CORES):
        us = slice(c * U_LOC, (c + 1) * U_LOC)
        bp = np.stack([bias_r[us], bias_z[us], b_ihn[us], b_hhn[us]], axis=-1)
        in_maps.append({
            "x_t": np.ascontiguousarray(x_all[us]),
            "h_t": np.ascontiguousarray(h_all[us]),
            "wih": np.ascontiguousarray(wihT[us]),
            "whh": np.ascontiguousarray(whhT[us]),
            "bia": np.ascontiguousarray(bp.transpose(1, 0, 2)),
        })
    return in_maps


def _run(in_maps, trace=False, **kw):
    _import_concourse()
    from concourse.bass_utils import run_bass_kernel_spmd

    nc = _build()
    return run_bass_kernel_spmd(nc, in_maps, list(range(N_CORES)), trace=trace, **kw)


def _assemble(res):
    out = np.concatenate([r["out_t"] for r in res.results], axis=0)  # (U, H, B)
    return np.ascontiguousarray(out.transpose(2, 0, 1))  # (B, U, H)


def kernel(**inputs):
    in_maps = _make_in_maps(
        np.asarray(inputs["inputs"]), np.asarray(inputs["hidden"]),
        np.asarray(inputs["w_ih"]), np.asarray(inputs["w_hh"]),
        np.asarray(inputs["b_ih"]), np.asarray(inputs["b_hh"]))
    return _assemble(_run(in_maps, trace=False))


def kernel_traced(inputs, **kw):
    """Test-harness entry: returns (output, BassKernelResults)."""
    in_maps = _make_in_maps(
        np.asarray(inputs["inputs"]), np.asarray(inputs["hidden"]),
        np.asarray(inputs["w_ih"]), np.asarray(inputs["w_hh"]),
        np.asarray(inputs["b_ih"]), np.asarray(inputs["b_hh"]))
    res = _run(in_maps, trace=True, **kw)
    return _assemble(res), res


# revision 12
# speedup vs baseline: 1.2966x; 1.2966x over previous
"""Grouped single-step GRU (16 independent GRU cells), Trainium2 Bass kernel.

Problem shapes (hardcoded): B=8192, U=16, I=H=128, fp32.
  gx = einsum("bui,ugi->bug", x, w_ih) + b_ih
  gh = einsum("buh,ugh->bug", h, w_hh) + b_hh
  r = sig(gx_r + gh_r); z = sig(gx_z + gh_z); n = tanh(gx_n + r * gh_n)
  out = (1-z)*n + z*h

Sharding: expert/unit-parallel — each of the 8 cores owns 2 units and the
full batch. This avoids replicating weights (only 0.75 MB of weights per
core) so per-core HBM traffic is ~24.75 MB, the problem's memory floor.

On-chip layout: gate dim (128) on partitions, batch on the free dim.
Host pre-transposes x/h to [u, feat, batch] so the contraction dim (feat)
lands on partitions for the TensorE moving operand; weights are
pre-transposed to [u, feat, 3H] to serve as stationary operands.
r/z gates accumulate their x- and h- matmuls in PSUM (2 matmuls, one bank);
the n gate keeps xn/hn separate (r gates hn before the sum).
"""

import os
import sys

import numpy as np

B, U, I, H = 8192, 16, 128, 128
N_CORES = 8
U_LOC = U // N_CORES  # units per core
BT = 512              # batch tile (free dim; one PSUM bank in fp32)
NT = B // BT

_CACHE = {}


def _import_concourse():
    try:
        import concourse.bass  # noqa: F401
    except ImportError:
        for p in ("/opt/trn_rl_repo", "/root/.axon_site/_ro/trn_rl_repo"):
            if os.path.isdir(p) and p not in sys.path:
                sys.path.insert(0, p)
        import concourse.bass  # noqa: F401


def _build():
    if "nc" in _CACHE:
        return _CACHE["nc"]
    _import_concourse()
    from contextlib import ExitStack

    import concourse.bacc as bacc
    import concourse.bass as bass
    import concourse.tile as tile
    from concourse import mybir

    f32 = mybir.dt.float32
    f32r = mybir.dt.float32r
    AFT = mybir.ActivationFunctionType
    ALU = mybir.AluOpType

    nc = bacc.Bacc(None, target_bir_lowering=False)
    x_t = nc.declare_dram_parameter("x_t", [U_LOC, I, B], f32r, isOutput=False)
    h_t = nc.declare_dram_parameter("h_t", [U_LOC, H, B], f32r, isOutput=False)
    wih = nc.declare_dram_parameter("wih", [U_LOC, I, 3 * H], f32r, isOutput=False)
    whh = nc.declare_dram_parameter("whh", [U_LOC, H, 3 * H], f32r, isOutput=False)
    bia = nc.declare_dram_parameter("bia", [H, U_LOC, 4], f32, isOutput=False)
    out_t = nc.declare_dram_parameter("out_t", [U_LOC, H, B], f32, isOutput=True)

    with ExitStack() as ctx:
        tc = ctx.enter_context(tile.TileContext(nc))
        wpool = ctx.enter_context(tc.tile_pool(name="w", bufs=1))
        xpool = ctx.enter_context(tc.tile_pool(name="x", bufs=4))
        hpool = ctx.enter_context(tc.tile_pool(name="h", bufs=4))
        tmp = ctx.enter_context(tc.tile_pool(name="tmp", bufs=3))
        opool = ctx.enter_context(tc.tile_pool(name="o", bufs=4))
        psum = ctx.enter_context(tc.tile_pool(name="psum", bufs=2, space="PSUM"))

        w_ih_sb = wpool.tile([I, U_LOC, 3 * H], f32r)
        w_hh_sb = wpool.tile([H, U_LOC, 3 * H], f32r)
        bias_sb = wpool.tile([H, U_LOC, 4], f32)
        nc.sync.dma_start(out=w_ih_sb, in_=wih[:].rearrange("u i g -> i u g"))
        nc.sync.dma_start(out=w_hh_sb, in_=whh[:].rearrange("u i g -> i u g"))
        nc.sync.dma_start(out=bias_sb, in_=bia[:])

        for u in range(U_LOC):
            for t in range(NT):
                bs = slice(t * BT, (t + 1) * BT)
                x_sb = xpool.tile([I, BT], f32r, tag="x")
                h_sb = hpool.tile([H, BT], f32r, tag="h")
                nc.sync.dma_start(out=x_sb, in_=x_t[u, :, bs])
                nc.sync.dma_start(out=h_sb, in_=h_t[u, :, bs])

                p_r = psum.tile([H, BT], f32, tag="pr")
                p_z = psum.tile([H, BT], f32, tag="pz")
                p_xn = psum.tile([H, BT], f32, tag="pxn")
                p_hn = psum.tile([H, BT], f32, tag="phn")

                # r/z: accumulate x- and h-contributions in PSUM.
                # fp32r: single-pass PE mode (4x faster than fp32's two
                # half-speed passes); accumulation stays fp32 in PSUM.
                xr_, hr_ = x_sb[:], h_sb[:]
                wi_, wh_ = w_ih_sb[:, u, :], w_hh_sb[:, u, :]
                nc.tensor.matmul(p_r, wi_[:, 0:H], xr_, start=True, stop=False)
                nc.tensor.matmul(p_r, wh_[:, 0:H], hr_, start=False, stop=True)
                nc.tensor.matmul(p_z, wi_[:, H:2 * H], xr_, start=True, stop=False)
                nc.tensor.matmul(p_z, wh_[:, H:2 * H], hr_, start=False, stop=True)
                nc.tensor.matmul(p_xn, wi_[:, 2 * H:], xr_, start=True, stop=True)
                nc.tensor.matmul(p_hn, wh_[:, 2 * H:], hr_, start=True, stop=True)

                r_sb = tmp.tile([H, BT], f32, tag="r")
                z_sb = tmp.tile([H, BT], f32, tag="z")
                nc.scalar.activation(out=r_sb, in_=p_r, func=AFT.Sigmoid,
                                     bias=bias_sb[:, u, 0:1])
                nc.scalar.activation(out=z_sb, in_=p_z, func=AFT.Sigmoid,
                                     bias=bias_sb[:, u, 1:2])
                # m = (hn + b_hhn) * r
                m_sb = tmp.tile([H, BT], f32, tag="m")
                nc.vector.scalar_tensor_tensor(
                    out=m_sb, in0=p_hn, scalar=bias_sb[:, u, 3:4], in1=r_sb,
                    op0=ALU.add, op1=ALU.mult)
                s_sb = tmp.tile([H, BT], f32, tag="s")
                nc.vector.tensor_add(s_sb, m_sb, p_xn)
                n_sb = tmp.tile([H, BT], f32, tag="n")
                nc.scalar.activation(out=n_sb, in_=s_sb, func=AFT.Tanh,
                                     bias=bias_sb[:, u, 2:3])
                # out = n + z*(h - n)
                d_sb = tmp.tile([H, BT], f32, tag="d")
                nc.vector.tensor_sub(d_sb, h_sb[:].bitcast(f32), n_sb)
                zd_sb = tmp.tile([H, BT], f32, tag="zd")
                nc.vector.tensor_mul(zd_sb, z_sb, d_sb)
                o_sb = opool.tile([H, BT], f32, tag="o")
                nc.vector.tensor_add(o_sb, n_sb, zd_sb)
                nc.sync.dma_start(out=out_t[u, :, bs], in_=o_sb)

    nc.compile()
    _CACHE["nc"] = nc
    return nc


def _make_in_maps(inputs, hidden, w_ih, w_hh, b_ih, b_hh):
    x_all = np.ascontiguousarray(inputs.transpose(1, 2, 0), dtype=np.float32)
    h_all = np.ascontiguousarray(hidden.transpose(1, 2, 0), dtype=np.float32)
    wihT = np.ascontiguousarray(w_ih.transpose(0, 2, 1), dtype=np.float32)
    whhT = np.ascontiguousarray(w_hh.transpose(0, 2, 1), dtype=np.float32)
    bias_r = (b_ih[:, :H] + b_hh[:, :H]).astype(np.float32)
    bias_z = (b_ih[:, H:2 * H] + b_hh[:, H:2 * H]).astype(np.float32)
    b_ihn = b_ih[:, 2 * H:].astype(np.float32)
    b_hhn = b_hh[:, 2 * H:].astype(np.float32)
    in_maps = []
    for c in range(N_CORES):
        us = slice(c * U_LOC, (c + 1) * U_LOC)
        bp = np.stack([bias_r[us], bias_z[us], b_ihn[us], b_hhn[us]], axis=-1)
        in_maps.append({
            "x_t": np.ascontiguousarray(x_all[us]),
            "h_t": np.ascontiguousarray(h_all[us]),
            "wih": np.ascontiguousarray(wihT[us]),
            "whh": np.ascontiguousarray(whhT[us]),
            "bia": np.ascontiguousarray(bp.transpose(1, 0, 2)),
        })
    return in_maps


def _run(in_maps, trace=False, **kw):
    _import_concourse()
    from concourse.bass_utils import run_bass_kernel_spmd

    nc = _build()
    return run_bass_kernel_spmd(nc, in_maps, list(range(N_CORES)), trace=trace, **kw)


def _assemble(res):
    out = np.concatenate([r["out_t"] for r in res.results], axis=0)  # (U, H, B)
    return np.ascontiguousarray(out.transpose(2, 0, 1))  # (B, U, H)


def kernel(**inputs):
    in_maps = _make_in_maps(
        np.asarray(inputs["inputs"]), np.asarray(inputs["hidden"]),
        np.asarray(inputs["w_ih"]), np.asarray(inputs["w_hh"]),
        np.asarray(inputs["b_ih"]), np.asarray(inputs["b_hh"]))
    return _assemble(_run(in_maps, trace=False))


def kernel_traced(inputs, **kw):
    """Test-harness entry: returns (output, BassKernelResults)."""
    in_maps = _make_in_maps(
        np.asarray(inputs["inputs"]), np.asarray(inputs["hidden"]),
        np.asarray(inputs["w_ih"]), np.asarray(inputs["w_hh"]),
        np.asarray(inputs["b_ih"]), np.asarray(inputs["b_hh"]))
    res = _run(in_maps, trace=True, **kw)
    return _assemble(res), res


# revision 13
# speedup vs baseline: 1.3550x; 1.0451x over previous
"""Grouped single-step GRU (16 independent GRU cells), Trainium2 Bass kernel.

Problem shapes (hardcoded): B=8192, U=16, I=H=128, fp32.
  gx = einsum("bui,ugi->bug", x, w_ih) + b_ih
  gh = einsum("buh,ugh->bug", h, w_hh) + b_hh
  r = sig(gx_r + gh_r); z = sig(gx_z + gh_z); n = tanh(gx_n + r * gh_n)
  out = (1-z)*n + z*h

Sharding: expert/unit-parallel — each of the 8 cores owns 2 units and the
full batch. This avoids replicating weights (only 0.75 MB of weights per
core) so per-core HBM traffic is ~24.75 MB, the problem's memory floor.

On-chip layout: gate dim (128) on partitions, batch on the free dim.
Host pre-transposes x/h to [u, feat, batch] so the contraction dim (feat)
lands on partitions for the TensorE moving operand; weights are
pre-transposed to [u, feat, 3H] to serve as stationary operands.
r/z gates accumulate their x- and h- matmuls in PSUM (2 matmuls, one bank);
the n gate keeps xn/hn separate (r gates hn before the sum).
"""

import os
import sys

import numpy as np

B, U, I, H = 8192, 16, 128, 128
N_CORES = 8
U_LOC = U // N_CORES  # units per core
BT = 512              # batch tile (free dim; one PSUM bank in fp32)
NT = B // BT

_CACHE = {}


def _import_concourse():
    try:
        import concourse.bass  # noqa: F401
    except ImportError:
        for p in ("/opt/trn_rl_repo", "/root/.axon_site/_ro/trn_rl_repo"):
            if os.path.isdir(p) and p not in sys.path:
                sys.path.insert(0, p)
        import concourse.bass  # noqa: F401


def _build():
    if "nc" in _CACHE:
        return _CACHE["nc"]
    _import_concourse()
    from contextlib import ExitStack

    import concourse.bacc as bacc
    import concourse.bass as bass
    import concourse.tile as tile
    from concourse import mybir

    f32 = mybir.dt.float32
    f32r = mybir.dt.float32r
    AFT = mybir.ActivationFunctionType
    ALU = mybir.AluOpType

    nc = bacc.Bacc(None, target_bir_lowering=False)
    x_t = nc.declare_dram_parameter("x_t", [U_LOC, I, B], f32r, isOutput=False)
    h_t = nc.declare_dram_parameter("h_t", [U_LOC, H, B], f32r, isOutput=False)
    wih = nc.declare_dram_parameter("wih", [U_LOC, I, 3 * H], f32r, isOutput=False)
    whh = nc.declare_dram_parameter("whh", [U_LOC, H, 3 * H], f32r, isOutput=False)
    bia = nc.declare_dram_parameter("bia", [H, U_LOC, 4], f32, isOutput=False)
    out_t = nc.declare_dram_parameter("out_t", [U_LOC, H, B], f32, isOutput=True)

    with ExitStack() as ctx:
        tc = ctx.enter_context(tile.TileContext(nc))
        wpool = ctx.enter_context(tc.tile_pool(name="w", bufs=1))
        xpool = ctx.enter_context(tc.tile_pool(name="x", bufs=6))
        hpool = ctx.enter_context(tc.tile_pool(name="h", bufs=6))
        tmp = ctx.enter_context(tc.tile_pool(name="tmp", bufs=4))
        opool = ctx.enter_context(tc.tile_pool(name="o", bufs=6))
        psum = ctx.enter_context(tc.tile_pool(name="psum", bufs=2, space="PSUM"))

        w_ih_sb = wpool.tile([I, U_LOC, 3 * H], f32r)
        w_hh_sb = wpool.tile([H, U_LOC, 3 * H], f32r)
        bias_sb = wpool.tile([H, U_LOC, 4], f32)
        nc.sync.dma_start(out=w_ih_sb, in_=wih[:].rearrange("u i g -> i u g"))
        nc.sync.dma_start(out=w_hh_sb, in_=whh[:].rearrange("u i g -> i u g"))
        nc.sync.dma_start(out=bias_sb, in_=bia[:])

        for u in range(U_LOC):
            for t in range(NT):
                bs = slice(t * BT, (t + 1) * BT)
                x_sb = xpool.tile([I, BT], f32r, tag="x")
                h_sb = hpool.tile([H, BT], f32r, tag="h")
                nc.sync.dma_start(out=x_sb, in_=x_t[u, :, bs])
                nc.sync.dma_start(out=h_sb, in_=h_t[u, :, bs])

                p_r = psum.tile([H, BT], f32, tag="pr")
                p_z = psum.tile([H, BT], f32, tag="pz")
                p_xn = psum.tile([H, BT], f32, tag="pxn")
                p_hn = psum.tile([H, BT], f32, tag="phn")

                # r/z: accumulate x- and h-contributions in PSUM.
                # fp32r: single-pass PE mode (4x faster than fp32's two
                # half-speed passes); accumulation stays fp32 in PSUM.
                xr_, hr_ = x_sb[:], h_sb[:]
                wi_, wh_ = w_ih_sb[:, u, :], w_hh_sb[:, u, :]
                nc.tensor.matmul(p_r, wi_[:, 0:H], xr_, start=True, stop=False)
                nc.tensor.matmul(p_r, wh_[:, 0:H], hr_, start=False, stop=True)
                nc.tensor.matmul(p_z, wi_[:, H:2 * H], xr_, start=True, stop=False)
                nc.tensor.matmul(p_z, wh_[:, H:2 * H], hr_, start=False, stop=True)
                nc.tensor.matmul(p_xn, wi_[:, 2 * H:], xr_, start=True, stop=True)
                nc.tensor.matmul(p_hn, wh_[:, 2 * H:], hr_, start=True, stop=True)

                r_sb = tmp.tile([H, BT], f32, tag="r")
                z_sb = tmp.tile([H, BT], f32, tag="z")
                nc.scalar.activation(out=r_sb, in_=p_r, func=AFT.Sigmoid,
                                     bias=bias_sb[:, u, 0:1])
                nc.scalar.activation(out=z_sb, in_=p_z, func=AFT.Sigmoid,
                                     bias=bias_sb[:, u, 1:2])
                # m = (hn + b_hhn) * r
                m_sb = tmp.tile([H, BT], f32, tag="m")
                nc.vector.scalar_tensor_tensor(
                    out=m_sb, in0=p_hn, scalar=bias_sb[:, u, 3:4], in1=r_sb,
                    op0=ALU.add, op1=ALU.mult)
                s_sb = tmp.tile([H, BT], f32, tag="s")
                nc.vector.tensor_add(s_sb, m_sb, p_xn)
                n_sb = tmp.tile([H, BT], f32, tag="n")
                nc.scalar.activation(out=n_sb, in_=s_sb, func=AFT.Tanh,
                                     bias=bias_sb[:, u, 2:3])
                # out = n + z*(h - n)
                d_sb = tmp.tile([H, BT], f32, tag="d")
                nc.gpsimd.tensor_sub(d_sb, h_sb[:].bitcast(f32), n_sb)
                zd_sb = tmp.tile([H, BT], f32, tag="zd")
                nc.vector.tensor_mul(zd_sb, z_sb, d_sb)
                o_sb = opool.tile([H, BT], f32, tag="o")
                nc.vector.tensor_add(o_sb, n_sb, zd_sb)
                nc.sync.dma_start(out=out_t[u, :, bs], in_=o_sb)

    nc.compile()
    _CACHE["nc"] = nc
    return nc


def _make_in_maps(inputs, hidden, w_ih, w_hh, b_ih, b_hh):
    x_all = np.ascontiguousarray(inputs.transpose(1, 2, 0), dtype=np.float32)
    h_all = np.ascontiguousarray(hidden.transpose(1, 2, 0), dtype=np.float32)
    wihT = np.ascontiguousarray(w_ih.transpose(0, 2, 1), dtype=np.float32)
    whhT = np.ascontiguousarray(w_hh.transpose(0, 2, 1), dtype=np.float32)
    bias_r = (b_ih[:, :H] + b_hh[:, :H]).astype(np.float32)
    bias_z = (b_ih[:, H:2 * H] + b_hh[:, H:2 * H]).astype(np.float32)
    b_ihn = b_ih[:, 2 * H:].astype(np.float32)
    b_hhn = b_hh[:, 2 * H:].astype(np.float32)
    in_maps = []
    for c in range(N_CORES):
        us = slice(c * U_LOC, (c + 1) * U_LOC)
        bp = np.stack([bias_r[us], bias_z[us], b_ihn[us], b_hhn[us]], axis=-1)
        in_maps.append({
            "x_t": np.ascontiguousarray(x_all[us]),
            "h_t": np.ascontiguousarray(h_all[us]),
            "wih": np.ascontiguousarray(wihT[us]),
            "whh": np.ascontiguousarray(whhT[us]),
            "bia": np.ascontiguousarray(bp.transpose(1, 0, 2)),
        })
    return in_maps


def _run(in_maps, trace=False, **kw):
    _import_concourse()
    from concourse.bass_utils import run_bass_kernel_spmd

    nc = _build()
    return run_bass_kernel_spmd(nc, in_maps, list(range(N_CORES)), trace=trace, **kw)


def _assemble(res):
    out = np.concatenate([r["out_t"] for r in res.results], axis=0)  # (U, H, B)
    return np.ascontiguousarray(out.transpose(2, 0, 1))  # (B, U, H)


def kernel(**inputs):
    in_maps = _make_in_maps(
        np.asarray(inputs["inputs"]), np.asarray(inputs["hidden"]),
        np.asarray(inputs["w_ih"]), np.asarray(inputs["w_hh"]),
        np.asarray(inputs["b_ih"]), np.asarray(inputs["b_hh"]))
    return _assemble(_run(in_maps, trace=False))


def kernel_traced(inputs, **kw):
    """Test-harness entry: returns (output, BassKernelResults)."""
    in_maps = _make_in_maps(
        np.asarray(inputs["inputs"]), np.asarray(inputs["hidden"]),
        np.asarray(inputs["w_ih"]), np.asarray(inputs["w_hh"]),
        np.asarray(inputs["b_ih"]), np.asarray(inputs["b_hh"]))
    res = _run(in_maps, trace=True, **kw)
    return _assemble(res), res


# revision 14
# speedup vs baseline: 1.3754x; 1.0151x over previous
"""Grouped single-step GRU (16 independent GRU cells), Trainium2 Bass kernel.

Problem shapes (hardcoded): B=8192, U=16, I=H=128, fp32.
  gx = einsum("bui,ugi->bug", x, w_ih) + b_ih
  gh = einsum("buh,ugh->bug", h, w_hh) + b_hh
  r = sig(gx_r + gh_r); z = sig(gx_z + gh_z); n = tanh(gx_n + r * gh_n)
  out = (1-z)*n + z*h

Sharding: expert/unit-parallel — each of the 8 cores owns 2 units and the
full batch. This avoids replicating weights (only 0.75 MB of weights per
core) so per-core HBM traffic is ~24.75 MB, the problem's memory floor.

On-chip layout: gate dim (128) on partitions, batch on the free dim.
Host pre-transposes x/h to [u, feat, batch] so the contraction dim (feat)
lands on partitions for the TensorE moving operand; weights are
pre-transposed to [u, feat, 3H] to serve as stationary operands.
r/z gates accumulate their x- and h- matmuls in PSUM (2 matmuls, one bank);
the n gate keeps xn/hn separate (r gates hn before the sum).
"""

import os
import sys

import numpy as np

B, U, I, H = 8192, 16, 128, 128
N_CORES = 8
U_LOC = U // N_CORES  # units per core
BT = 512              # batch tile (free dim; one PSUM bank in fp32)
NT = B // BT

_CACHE = {}


def _import_concourse():
    try:
        import concourse.bass  # noqa: F401
    except ImportError:
        for p in ("/opt/trn_rl_repo", "/root/.axon_site/_ro/trn_rl_repo"):
            if os.path.isdir(p) and p not in sys.path:
                sys.path.insert(0, p)
        import concourse.bass  # noqa: F401


def _build():
    if "nc" in _CACHE:
        return _CACHE["nc"]
    _import_concourse()
    from contextlib import ExitStack

    import concourse.bacc as bacc
    import concourse.bass as bass
    import concourse.tile as tile
    from concourse import mybir

    f32 = mybir.dt.float32
    f32r = mybir.dt.float32r
    AFT = mybir.ActivationFunctionType
    ALU = mybir.AluOpType

    nc = bacc.Bacc(None, target_bir_lowering=False)
    x_t = nc.declare_dram_parameter("x_t", [U_LOC, I, B], f32r, isOutput=False)
    h_t = nc.declare_dram_parameter("h_t", [U_LOC, H, B], f32r, isOutput=False)
    wih = nc.declare_dram_parameter("wih", [U_LOC, I, 3 * H], f32r, isOutput=False)
    whh = nc.declare_dram_parameter("whh", [U_LOC, H, 3 * H], f32r, isOutput=False)
    bia = nc.declare_dram_parameter("bia", [H, U_LOC, 4], f32, isOutput=False)
    out_t = nc.declare_dram_parameter("out_t", [U_LOC, H, B], f32, isOutput=True)

    with ExitStack() as ctx:
        tc = ctx.enter_context(tile.TileContext(nc))
        wpool = ctx.enter_context(tc.tile_pool(name="w", bufs=1))
        xpool = ctx.enter_context(tc.tile_pool(name="x", bufs=6))
        hpool = ctx.enter_context(tc.tile_pool(name="h", bufs=6))
        tmp = ctx.enter_context(tc.tile_pool(name="tmp", bufs=4))
        opool = ctx.enter_context(tc.tile_pool(name="o", bufs=6))
        psum = ctx.enter_context(tc.tile_pool(name="psum", bufs=2, space="PSUM"))

        w_ih_sb = wpool.tile([I, U_LOC, 3 * H], f32r)
        w_hh_sb = wpool.tile([H, U_LOC, 3 * H], f32r)
        bias_sb = wpool.tile([H, U_LOC, 4], f32)
        nc.sync.dma_start(out=w_ih_sb, in_=wih[:].rearrange("u i g -> i u g"))
        nc.sync.dma_start(out=w_hh_sb, in_=whh[:].rearrange("u i g -> i u g"))
        nc.sync.dma_start(out=bias_sb, in_=bia[:])

        for u in range(U_LOC):
            for t in range(NT):
                bs = slice(t * BT, (t + 1) * BT)
                x_sb = xpool.tile([I, BT], f32r, tag="x")
                h_sb = hpool.tile([H, BT], f32r, tag="h")
                nc.sync.dma_start(out=x_sb, in_=x_t[u, :, bs])
                nc.sync.dma_start(out=h_sb, in_=h_t[u, :, bs])

                p_r = psum.tile([H, BT], f32, tag="pr")
                p_z = psum.tile([H, BT], f32, tag="pz")
                p_xn = psum.tile([H, BT], f32, tag="pxn")
                p_hn = psum.tile([H, BT], f32, tag="phn")

                # r/z: accumulate x- and h-contributions in PSUM.
                # fp32r: single-pass PE mode (4x faster than fp32's two
                # half-speed passes); accumulation stays fp32 in PSUM.
                xr_, hr_ = x_sb[:], h_sb[:]
                wi_, wh_ = w_ih_sb[:, u, :], w_hh_sb[:, u, :]
                nc.tensor.matmul(p_r, wi_[:, 0:H], xr_, start=True, stop=False)
                nc.tensor.matmul(p_r, wh_[:, 0:H], hr_, start=False, stop=True)
                nc.tensor.matmul(p_z, wi_[:, H:2 * H], xr_, start=True, stop=False)
                nc.tensor.matmul(p_z, wh_[:, H:2 * H], hr_, start=False, stop=True)
                nc.tensor.matmul(p_xn, wi_[:, 2 * H:], xr_, start=True, stop=True)
                nc.tensor.matmul(p_hn, wh_[:, 2 * H:], hr_, start=True, stop=True)

                r_sb = tmp.tile([H, BT], f32, tag="r")
                z_sb = tmp.tile([H, BT], f32, tag="z")
                nc.scalar.activation(out=r_sb, in_=p_r, func=AFT.Sigmoid,
                                     bias=bias_sb[:, u, 0:1])
                nc.scalar.activation(out=z_sb, in_=p_z, func=AFT.Sigmoid,
                                     bias=bias_sb[:, u, 1:2])
                # m = (hn + b_hhn) * r
                m_sb = tmp.tile([H, BT], f32, tag="m")
                nc.vector.scalar_tensor_tensor(
                    out=m_sb, in0=p_hn, scalar=bias_sb[:, u, 3:4], in1=r_sb,
                    op0=ALU.add, op1=ALU.mult)
                s_sb = tmp.tile([H, BT], f32, tag="s")
                nc.vector.tensor_add(s_sb, m_sb, p_xn)
                n_sb = tmp.tile([H, BT], f32, tag="n")
                nc.scalar.activation(out=n_sb, in_=s_sb, func=AFT.Tanh,
                                     bias=bias_sb[:, u, 2:3])
                # out = n + z*(h - n)
                d_sb = tmp.tile([H, BT], f32, tag="d")
                nc.gpsimd.tensor_sub(d_sb, h_sb[:].bitcast(f32), n_sb)
                zd_sb = tmp.tile([H, BT], f32, tag="zd")
                nc.vector.tensor_mul(zd_sb, z_sb, d_sb)
                o_sb = opool.tile([H, BT], f32, tag="o")
                nc.gpsimd.tensor_add(o_sb, n_sb, zd_sb)
                nc.sync.dma_start(out=out_t[u, :, bs], in_=o_sb)

    nc.compile()
    _CACHE["nc"] = nc
    return nc


def _make_in_maps(inputs, hidden, w_ih, w_hh, b_ih, b_hh):
    x_all = np.ascontiguousarray(inputs.transpose(1, 2, 0), dtype=np.float32)
    h_all = np.ascontiguousarray(hidden.transpose(1, 2, 0), dtype=np.float32)
    wihT = np.ascontiguousarray(w_ih.transpose(0, 2, 1), dtype=np.float32)
    whhT = np.ascontiguousarray(w_hh.transpose(0, 2, 1), dtype=np.float32)
    bias_r = (b_ih[:, :H] + b_hh[:, :H]).astype(np.float32)
    bias_z = (b_ih[:, H:2 * H] + b_hh[:, H:2 * H]).astype(np.float32)
    b_ihn = b_ih[:, 2 * H:].astype(np.float32)
    b_hhn = b_hh[:, 2 * H:].astype(np.float32)
    in_maps = []
    for c in range(N_CORES):
        us = slice(c * U_LOC, (c + 1) * U_LOC)
        bp = np.stack([bias_r[us], bias_z[us], b_ihn[us], b_hhn[us]], axis=-1)
        in_maps.append({
            "x_t": np.ascontiguousarray(x_all[us]),
            "h_t": np.ascontiguousarray(h_all[us]),
            "wih": np.ascontiguousarray(wihT[us]),
            "whh": np.ascontiguousarray(whhT[us]),
            "bia": np.ascontiguousarray(bp.transpose(1, 0, 2)),
        })
    return in_maps


def _run(in_maps, trace=False, **kw):
    _import_concourse()
    from concourse.bass_utils import run_bass_kernel_spmd

    nc = _build()
    return run_bass_kernel_spmd(nc, in_maps, list(range(N_CORES)), trace=trace, **kw)


def _assemble(res):
    out = np.concatenate([r["out_t"] for r in res.results], axis=0)  # (U, H, B)
    return np.ascontiguousarray(out.transpose(2, 0, 1))  # (B, U, H)


def kernel(**inputs):
    in_maps = _make_in_maps(
        np.asarray(inputs["inputs"]), np.asarray(inputs["hidden"]),
        np.asarray(inputs["w_ih"]), np.asarray(inputs["w_hh"]),
        np.asarray(inputs["b_ih"]), np.asarray(inputs["b_hh"]))
    return _assemble(_run(in_maps, trace=False))


def kernel_traced(inputs, **kw):
    """Test-harness entry: returns (output, BassKernelResults)."""
    in_maps = _make_in_maps(
        np.asarray(inputs["inputs"]), np.asarray(inputs["hidden"]),
        np.asarray(inputs["w_ih"]), np.asarray(inputs["w_hh"]),
        np.asarray(inputs["b_ih"]), np.asarray(inputs["b_hh"]))
    res = _run(in_maps, trace=True, **kw)
    return _assemble(res), res
